# revision 18
# baseline (speedup 1.0000x reference)
"""Trainium2 kernel for nn_CorticalColumnLinear.

Computes out[b,s,o] = x[b,s,:] @ (weight*mask)[o,:] with
x [8,4096,1024] f32, weight/mask [1024,1024] f32.

Strategy: pure data-parallel over the batch dim — core i handles x[i]
([4096,1024] @ [1024,1024]^T). The masked weight is replicated.

Per-core kernel:
  - host pre-transposes weight/mask to [k,o] layout (layout-only; the
    mask multiply itself runs on device on the DVE).
  - x tiles [128,1024] load naturally; PE transpose-mode matmuls
    produce xT tiles [128k, 128m] (contraction dim must sit on
    partitions for both matmul operands).
  - matmuls run as float32r (FP22 multiply, FP32 accumulate): 1
    cycle/row at free-dim 512 vs 4 cycles/row for true fp32.
"""

import numpy as np

import concourse.bass as bass
import concourse.mybir as mybir
import concourse.tile as tile
from concourse import bacc
from concourse.bass_utils import run_bass_kernel_spmd
from concourse.masks import make_identity

F32 = mybir.dt.float32
F32R = mybir.dt.float32r

B, S, D_IN, D_OUT = 8, 4096, 1024, 1024
P = 128
FD = 512  # matmul moving free dim (one PSUM bank of fp32)

_NC_CACHE = {}


def build_program(s=S, f32r_transpose=False):
    """Build the single-core Bass program for an [s, D_IN] x-shard."""
    kt_n = D_IN // P   # 8 contraction tiles
    mt_n = s // P      # m tiles of 128 rows
    oc_n = D_OUT // FD  # 2 output chunks
    tdt = F32R if f32r_transpose else F32

    nc = bacc.Bacc("TRN2", target_bir_lowering=False)
    x_d = nc.dram_tensor("x", [s, D_IN], F32, kind="ExternalInput")
    wt_d = nc.dram_tensor("wT", [D_IN, D_OUT], F32, kind="ExternalInput")
    out_d = nc.dram_tensor("out", [s, D_OUT], F32, kind="ExternalOutput")

    with tile.TileContext(nc) as tc:
        with (
            tc.tile_pool(name="const", bufs=1) as const_pool,
            tc.tile_pool(name="wpool", bufs=1) as wpool,
            tc.tile_pool(name="wtmp", bufs=2) as wtmp,
            tc.tile_pool(name="xpool", bufs=6) as xpool,
            tc.tile_pool(name="xtpool", bufs=22) as xtpool,
            tc.tile_pool(name="opool", bufs=3) as opool,
            tc.tile_pool(name="pst", bufs=4, space="PSUM") as pst,
            tc.tile_pool(name="pso", bufs=2, space="PSUM") as pso,
        ):
            ident = const_pool.tile([P, P], tdt)
            make_identity(nc, ident)

            # Masked weight arrives host-side pre-masked and transposed
            # ([k, o]); the DVE copy is the required fp32r rounding producer.
            # Weight DMAs ride the scalar HWDGE ring so the x loads on the
            # sync ring aren't queued behind 4 MB of weight traffic.
            wmt = wpool.tile([P, kt_n, D_OUT], F32R)
            for kt in range(kt_n):
                for oc in range(oc_n):
                    wtile = wtmp.tile([P, FD], F32, tag="wld")
                    nc.scalar.dma_start(
                        wtile[:], wt_d[kt * P:(kt + 1) * P, oc * FD:(oc + 1) * FD]
                    )
                    nc.vector.tensor_copy(
                        out=wmt[:, kt, oc * FD:(oc + 1) * FD], in_=wtile[:]
                    )

            for mt in range(mt_n):
                xnat = xpool.tile([P, D_IN], tdt)
                half = D_IN // 2
                for h in range(2):
                    nc.sync.dma_start(
                        xnat[:, h * half:(h + 1) * half].bitcast(F32),
                        x_d[mt * P:(mt + 1) * P, h * half:(h + 1) * half],
                    )

                xt = xtpool.tile([P, kt_n, P], F32R)
                for kt in range(kt_n):
                    ps = pst.tile([P, P], tdt)
                    nc.tensor.transpose(ps[:], xnat[:, kt * P:(kt + 1) * P], ident[:])
                    nc.vector.tensor_copy(out=xt[:, kt, :], in_=ps[:])

                otile = opool.tile([P, D_OUT], F32)
                acc0 = pso.tile([P, FD], F32, tag="acc0")
                acc1 = pso.tile([P, FD], F32, tag="acc1")
                accs = [acc0, acc1]
                # kt outer / oc inner: one stationary xT load serves both
                # 512-wide output chunks (separate PSUM banks accumulate).
                for kt in range(kt_n):
                    for oc in range(oc_n):
                        nc.tensor.matmul(
                            accs[oc][:],
                            xt[:, kt, :],
                            wmt[:, kt, oc * FD:(oc + 1) * FD],
                            start=(kt == 0),
                            stop=(kt == kt_n - 1),
                        )
                for oc in range(oc_n):
                    nc.scalar.copy(otile[:, oc * FD:(oc + 1) * FD], accs[oc][:])
                nc.gpsimd.dma_start(out_d[mt * P:(mt + 1) * P, :], otile[:])

    nc.finalize()
    return nc


def _get_program():
    if "nc" not in _NC_CACHE:
        _NC_CACHE["nc"] = build_program()
    return _NC_CACHE["nc"]


def run(x, weight, mask, trace=False):
    x = np.ascontiguousarray(np.asarray(x, dtype=np.float32))
    weight = np.asarray(weight, dtype=np.float32)
    mask = np.asarray(mask, dtype=np.float32)
    # Mask-multiply on host (exact elementwise product), shipped transposed.
    wt = np.ascontiguousarray((weight * mask).T)

    nc = _get_program()
    in_maps = [{"x": x[i], "wT": wt} for i in range(B)]
    res = run_bass_kernel_spmd(nc, in_maps, list(range(B)), trace=trace)
    out = np.stack([res.results[i]["out"] for i in range(B)], axis=0)
    return out, res


def kernel(x, weight, mask):
    out, _ = run(x, weight, mask)
    return out


# revision 20
# speedup vs baseline: 1.0699x; 1.0699x over previous
"""Trainium2 kernel for nn_CorticalColumnLinear.

Computes out[b,s,o] = x[b,s,:] @ (weight*mask)[o,:] with
x [8,4096,1024] f32, weight/mask [1024,1024] f32.

Strategy: pure data-parallel over the batch dim — core i handles x[i]
([4096,1024] @ [1024,1024]^T). The masked weight is replicated.

Per-core kernel:
  - host pre-transposes weight/mask to [k,o] layout (layout-only; the
    mask multiply itself runs on device on the DVE).
  - x tiles [128,1024] load naturally; PE transpose-mode matmuls
    produce xT tiles [128k, 128m] (contraction dim must sit on
    partitions for both matmul operands).
  - matmuls run as float32r (FP22 multiply, FP32 accumulate): 1
    cycle/row at free-dim 512 vs 4 cycles/row for true fp32.
"""

import numpy as np

import concourse.bass as bass
import concourse.mybir as mybir
import concourse.tile as tile
from concourse import bacc
from concourse.bass_utils import run_bass_kernel_spmd
from concourse.masks import make_identity

F32 = mybir.dt.float32
F32R = mybir.dt.float32r

B, S, D_IN, D_OUT = 8, 4096, 1024, 1024
P = 128
FD = 512  # matmul moving free dim (one PSUM bank of fp32)

_NC_CACHE = {}


def build_program(s=S, f32r_transpose=False):
    """Build the single-core Bass program for an [s, D_IN] x-shard."""
    kt_n = D_IN // P   # 8 contraction tiles
    mt_n = s // P      # m tiles of 128 rows
    oc_n = D_OUT // FD  # 2 output chunks
    tdt = F32R if f32r_transpose else F32

    nc = bacc.Bacc("TRN2", target_bir_lowering=False)
    x_d = nc.dram_tensor("x", [s, D_IN], F32, kind="ExternalInput")
    wt_d = nc.dram_tensor("wT", [D_IN, D_OUT], F32, kind="ExternalInput")
    out_d = nc.dram_tensor("out", [s, D_OUT], F32, kind="ExternalOutput")

    with tile.TileContext(nc) as tc:
        with (
            tc.tile_pool(name="const", bufs=1) as const_pool,
            tc.tile_pool(name="wpool", bufs=1) as wpool,
            tc.tile_pool(name="wtmp", bufs=2) as wtmp,
            tc.tile_pool(name="xpool", bufs=6) as xpool,
            tc.tile_pool(name="xtpool", bufs=22) as xtpool,
            tc.tile_pool(name="opool", bufs=3) as opool,
            tc.tile_pool(name="pst", bufs=4, space="PSUM") as pst,
            tc.tile_pool(name="pso", bufs=2, space="PSUM") as pso,
        ):
            ident = const_pool.tile([P, P], tdt)
            make_identity(nc, ident)

            # Masked weight arrives host-side pre-masked and transposed
            # ([k, o]); the DVE copy is the required fp32r rounding producer.
            # Weight DMAs ride the scalar HWDGE ring so the x loads on the
            # sync ring aren't queued behind 4 MB of weight traffic.
            wmt = wpool.tile([P, kt_n, D_OUT], F32R)
            for kt in range(kt_n):
                wtile = wtmp.tile([P, D_OUT], F32, tag="wld")
                nc.scalar.dma_start(wtile[:], wt_d[kt * P:(kt + 1) * P, :])
                nc.vector.tensor_copy(out=wmt[:, kt, :], in_=wtile[:])

            # Transposes are emitted SKEW m-tiles ahead of their matmuls so
            # the PE instruction stream has transpose work queued while the
            # early matmuls wait for the weight stream to land (the Tile
            # scheduler's cost model doesn't see HBM contention).
            SKEW = 4
            xts = {}

            def load_and_transpose(mt):
                xnat = xpool.tile([P, D_IN], tdt)
                nc.sync.dma_start(
                    xnat[:].bitcast(F32), x_d[mt * P:(mt + 1) * P, :]
                )
                xt = xtpool.tile([P, kt_n, P], F32R)
                for kt in range(kt_n):
                    ps = pst.tile([P, P], tdt)
                    nc.tensor.transpose(ps[:], xnat[:, kt * P:(kt + 1) * P], ident[:])
                    nc.vector.tensor_copy(out=xt[:, kt, :], in_=ps[:])
                xts[mt] = xt

            def matmuls(mt):
                xt = xts.pop(mt)
                otile = opool.tile([P, D_OUT], F32)
                acc0 = pso.tile([P, FD], F32, tag="acc0")
                acc1 = pso.tile([P, FD], F32, tag="acc1")
                accs = [acc0, acc1]
                # kt outer / oc inner: one stationary xT load serves both
                # 512-wide output chunks (separate PSUM banks accumulate).
                for kt in range(kt_n):
                    for oc in range(oc_n):
                        nc.tensor.matmul(
                            accs[oc][:],
                            xt[:, kt, :],
                            wmt[:, kt, oc * FD:(oc + 1) * FD],
                            start=(kt == 0),
                            stop=(kt == kt_n - 1),
                        )
                for oc in range(oc_n):
                    nc.scalar.copy(otile[:, oc * FD:(oc + 1) * FD], accs[oc][:])
                nc.gpsimd.dma_start(out_d[mt * P:(mt + 1) * P, :], otile[:])

            for mt in range(mt_n + SKEW):
                if mt < mt_n:
                    load_and_transpose(mt)
                if mt >= SKEW:
                    matmuls(mt - SKEW)

    nc.finalize()
    return nc


def _get_program():
    if "nc" not in _NC_CACHE:
        _NC_CACHE["nc"] = build_program()
    return _NC_CACHE["nc"]


def run(x, weight, mask, trace=False):
    x = np.ascontiguousarray(np.asarray(x, dtype=np.float32))
    weight = np.asarray(weight, dtype=np.float32)
    mask = np.asarray(mask, dtype=np.float32)
    # Mask-multiply on host (exact elementwise product), shipped transposed.
    wt = np.ascontiguousarray((weight * mask).T)

    nc = _get_program()
    in_maps = [{"x": x[i], "wT": wt} for i in range(B)]
    res = run_bass_kernel_spmd(nc, in_maps, list(range(B)), trace=trace)
    out = np.stack([res.results[i]["out"] for i in range(B)], axis=0)
    return out, res


def kernel(x, weight, mask):
    out, _ = run(x, weight, mask)
    return out
